# revision 11
# baseline (speedup 1.0000x reference)
"""3-layer GAT on 8 TRN2 NeuronCores.

Strategy (1D vertex-cut, dst-sharded), v2 — dma_gather edge pass:
  * Nodes are permuted: degree-sorted, dealt round-robin to 8 cores, so each
    core owns a contiguous range of NC=6272 "new" node ids whose windows of
    128 consecutive ids have near-uniform in-degree.
  * Per (core, window) the edge list is laid out as K[w] "slots" x 128 dst
    rows (shared K schedule across cores, pad slots get logew=-1e30 and
    gather a real table row).
  * Edge gather uses gpsimd.dma_gather (Q7 SWDGE ucode, InstDMAGatherAnt):
    one instruction per <=8 slot columns (num_idxs = 128*cols <= 1024).
    idx position c*128+p lands at V[p, c, :] — exactly the slot grid.
    Tables are [npad, 256] bf16 (512B rows: h(128)|als(4)|junk pad) for
    layers 1/2 and [npad, 128] (256B rows: h3(64)|als3(1)|junk) for layer 3,
    because elem_size must be a multiple of 256B.
  * int16 index range trick: in_ap base = table row 17408, idx16 =
    table_row(src) - 17408 in [-17408, 32767]. The Q7 ucode processes
    mid-list negative idxs as signed offsets (HW-verified); only TRAILING
    negative idxs are trimmed. Host therefore guarantees the LAST idx of
    every chunk (slot (p=127, chunk-last-col)) is >= 0 by permuting
    partition 127's slot columns (pads have idx 0 and also qualify).
  * Per layer, per window: attention w = exp(leaky(als+ald)+log_ew),
    rhs = [V*w | w], K identity-lhsT matmuls accumulate [128, F+H] in PSUM
    (weighted segment-sum + denominator), normalize, bias(+ReLU).
  * Next-layer tables h'=relu(o)@W', als'=o@(W'.a) are produced per window
    (PE transpose + one matmul) and AllGather'd across cores between layers,
    split into two chunks (chunk-a issued mid-layer) to hide latency.
  * segment_max is omitted: logits are bounded, exp is safe in f32, and
    softmax is shift-invariant -> mathematically identical to the reference.

HW facts (measured this session / previous session — do not regress):
  * Pool-engine SWDGE descriptor generation is the kernel's hard floor:
    dma_gather = 503ns fixed + ~7.95ns/idx (vs indirect_dma_start 1.1us per
    128 rows = 11.3ns/row incl 310ns dispatch gap). ~8ns/row is the best
    any gather primitive achieves; HWDGE engines cannot execute indirect
    DMAs (device error), multi-column offset APs scramble on HW, and
    num_idxs >= 1792 in one dma_gather crashes the device (1024 is safe).
  * PSUM pool bufs beyond (3,2) crash the device; SBUF pools bufs=4 safe.
  * Run-to-run HW variance ~+/-25us.
"""
import numpy as np

# problem constants (hardcoded per harness contract)
N, E, IN, HID, HEADS, OUT = 50000, 800000, 256, 32, 4, 64
SLOPE = 0.2
CORES = 8
P = 128
BASE = 17408          # dma_gather in_ap base row (int16 range trick)
CHUNK_COLS = 8        # max slot columns per dma_gather (1024 idxs)


def ag_bounds(wpc):
    """Window boundaries of the table AllGather chunks (GA-aligned)."""
    ga = next(g for g in (7, 4, 2, 1) if wpc % g == 0)
    if wpc == 49:
        return [0, 28, 42, 49]
    half = ((wpc // ga + 1) // 2) * ga
    return [0, half, wpc]


# ----------------------------------------------------------------------------
# host-side schedule construction (index data only)
# ----------------------------------------------------------------------------
def build_schedule(src, dst, ew, n_nodes, npad, cores):
    """Returns node permutation + per-core slot arrays.

    perm: old->new node id (len npad); Ks: [W] slots per window;
    offs: [W+1] column offsets; idx16: [cores, 128, 8*S] int16 wrapped
    dma_gather indices; logew: [cores, 128, S] f32; chunks: list of
    (w, c0, cols, off16) per window-chunk.
    """
    nc_rows = npad // cores
    wpc = nc_rows // P
    src = np.asarray(src, np.int64)
    dst = np.asarray(dst, np.int64)
    ew = np.asarray(ew, np.float32)

    deg = np.bincount(dst, minlength=npad)
    order = np.argsort(-deg, kind="stable")          # ranks -> old id
    perm = np.empty(npad, np.int64)
    ranks = np.arange(npad)
    perm[order] = (ranks % cores) * nc_rows + ranks // cores

    # table rows are laid out chunk-major (chunk i = rows of window range
    # [bounds[i], bounds[i+1]) of every core, concatenated) so the
    # between-layer AllGathers split into several contiguous collectives,
    # the early big ones overlapping the producing layer's tail and only a
    # small final chunk's latency exposed at the layer boundary.
    bounds = ag_bounds(wpc)

    def table_row(n):
        c, r = n // nc_rows, n % nc_rows
        conds, vals = [], []
        for b0, b1 in zip(bounds, bounds[1:]):
            lo, rows = b0 * P, (b1 - b0) * P
            conds.append(r < b1 * P)
            vals.append(cores * lo + c * rows + (r - lo))
        return np.select(conds, vals)

    nsrc = perm[src]
    ndst = perm[dst]
    eorder = np.argsort(ndst, kind="stable")
    nsrc_s = nsrc[eorder]
    ndst_s = ndst[eorder]
    ew_s = ew[eorder]

    counts = np.bincount(ndst_s, minlength=npad)
    starts = np.zeros(npad + 1, np.int64)
    np.cumsum(counts, out=starts[1:])
    rank_in_dst = np.arange(len(ndst_s)) - starts[ndst_s]

    Ks = counts.reshape(cores, wpc, P).max(axis=(0, 2))
    Ks = np.maximum(Ks, 1).astype(np.int64)
    offs = np.zeros(wpc + 1, np.int64)
    np.cumsum(Ks, out=offs[1:])
    S = int(offs[-1])

    core_e = ndst_s // nc_rows
    loc = ndst_s % nc_rows
    w_e = loc // P
    p_e = loc % P
    col = offs[w_e] + rank_in_dst

    # slot grids: table-row per slot (pads -> BASE, logew -> -1e30)
    gtab = np.full((cores, P, S), BASE, np.int64)
    logew = np.full((cores, P, S), -1e30, np.float32)
    flat = (core_e * P + p_e) * S + col
    gtab.reshape(-1)[flat] = table_row(nsrc_s)
    with np.errstate(divide="ignore"):
        logew.reshape(-1)[flat] = np.log(np.maximum(ew_s, 0.0)).astype(np.float32)

    # chunking: <=CHUNK_COLS columns per dma_gather; the idx at list position
    # 128*cols-1 (= slot (127, chunk-last-col)) must be >= BASE. Fix by
    # permuting partition-127 columns within each window (pads qualify too).
    chunks = []
    off16 = 0
    for w in range(wpc):
        K = int(Ks[w])
        ends = []
        c0 = 0
        while c0 < K:
            cols = min(CHUNK_COLS, K - c0)
            chunks.append((w, c0, cols, off16))
            off16 += 8 * cols
            ends.append(c0 + cols - 1)
            c0 += cols
        o = int(offs[w])
        for c in range(cores):
            row = gtab[c, 127, o:o + K]
            lw = logew[c, 127, o:o + K]
            bad = [e for e in ends if row[e] < BASE]
            if bad:
                free = [j for j in range(K)
                        if row[j] >= BASE and j not in ends]
                assert len(free) >= len(bad), (
                    f"cannot fix chunk-final idx: core {c} window {w}")
                for e, j in zip(bad, free):
                    row[e], row[j] = row[j], row[e]
                    lw[e], lw[j] = lw[j], lw[e]

    # wrapped int16 idx arrays: per chunk, position i = c_local*128 + p
    idx16 = np.empty((cores, P, 8 * S), np.int16)
    for (w, c0, cols, o16) in chunks:
        o = int(offs[w])
        blk = gtab[:, :, o + c0:o + c0 + cols] - BASE       # [cores, P, cols]
        lst = blk.transpose(0, 2, 1).reshape(cores, cols * P)  # i = c*128+p
        wrap = lst.reshape(cores, cols * 8, 16).transpose(0, 2, 1)  # [c,16,8c]
        idx16[:, :, o16:o16 + 8 * cols] = np.tile(wrap, (1, 8, 1))
    return perm, Ks, offs, chunks, idx16, logew


def _np_bf16(x):
    import ml_dtypes
    return np.asarray(x, np.float32).astype(ml_dtypes.bfloat16)


# ----------------------------------------------------------------------------
# device program
# ----------------------------------------------------------------------------
def build_program(npad, Ks, offs, chunks, S, in_f, hid_heads, out_f, heads3,
                  cores, debug_w=None):
    """Build the SPMD Bacc program. Shapes:
      xTt   [NT, in_f, 128] bf16   (transposed x, node-tile blocks)
      w1cat [in_f//128, 128, hid_heads+8] bf16
      w2cat [hid_heads, hid_heads+8] bf16
      w3cat [hid_heads, out_f+8] bf16
      b1row/b2row [1, hid_heads] f32 ; b3row [1, out_f] f32
      idx16 [128, 8*S] int16 (wrapped dma_gather indices)
      logew [128, S] bf16
      out   [NC, out_f] f32 (per-core shard)
    """
    import concourse.bacc as bacc
    import concourse.bass as bass
    import concourse.mybir as mybir
    from concourse.library_config import mlp
    from concourse.masks import make_identity
    from concourse.tile import TileContext

    F32, BF16, I16 = mybir.dt.float32, mybir.dt.bfloat16, mybir.dt.int16
    AF = mybir.ActivationFunctionType
    ALU = mybir.AluOpType

    nc_rows = npad // cores
    wpc = nc_rows // P
    ic = in_f // P                       # input chunk count (2)
    D = hid_heads                         # 128
    H = HEADS
    TW, TW3 = 256, 128                    # table row widths (elems, bf16)

    nc = bacc.Bacc("TRN2", target_bir_lowering=False, debug=False,
                   enable_asserts=False, num_devices=cores)

    GA = next(g for g in (7, 4, 2, 1) if wpc % g == 0)
    ngrp = wpc // GA
    xTt = nc.dram_tensor("xTt", [ngrp, P, GA * in_f], BF16,
                         kind="ExternalInput")
    w1cat = nc.dram_tensor("w1cat", [ic, P, D + 8], BF16, kind="ExternalInput")
    w2cat = nc.dram_tensor("w2cat", [D, D + 8], BF16, kind="ExternalInput")
    w3cat = nc.dram_tensor("w3cat", [D, out_f + 8], BF16, kind="ExternalInput")
    b1row = nc.dram_tensor("b1row", [1, D], F32, kind="ExternalInput")
    b2row = nc.dram_tensor("b2row", [1, D], F32, kind="ExternalInput")
    b3row = nc.dram_tensor("b3row", [1, out_f], F32, kind="ExternalInput")
    idx16_d = nc.dram_tensor("idx16", [P, 8 * S], I16, kind="ExternalInput")
    logew_d = nc.dram_tensor("logew", [P, S], BF16, kind="ExternalInput")
    out_d = nc.dram_tensor("out", [nc_rows, out_f], F32, kind="ExternalOutput")
    dbgV = None
    if debug_w is not None:
        dbgV = nc.dram_tensor("dbgV", [P, int(Ks[debug_w]) * 256], BF16,
                              kind="ExternalOutput")
    dbgo1 = dbgo2 = None
    if debug_w is not None:
        dbgo1 = nc.dram_tensor("dbgo1", [nc_rows, D], F32, kind="ExternalOutput")
        dbgo2 = nc.dram_tensor("dbgo2", [nc_rows, D], F32, kind="ExternalOutput")

    # internal DRAM
    h1tab = nc.dram_tensor("h1tab", [npad, TW], BF16, addr_space="Shared")
    h2tab = nc.dram_tensor("h2tab", [npad, TW], BF16, addr_space="Shared")
    h3tab = nc.dram_tensor("h3tab", [npad, TW3], BF16, addr_space="Shared")

    rg = [list(range(cores))]
    bounds = ag_bounds(wpc)
    nch = len(bounds) - 1

    def ag_bufs(name, width):
        return [nc.dram_tensor(f"{name}{i}", [(b1 - b0) * P, width], BF16)
                for i, (b0, b1) in enumerate(zip(bounds, bounds[1:]))]

    ag1 = ag_bufs("ag1h", TW)
    ag2 = ag_bufs("ag2h", TW)
    ag3 = ag_bufs("ag3h", TW3)

    def ag_issue(agl, tab, i):
        lo, rows = bounds[i] * P, (bounds[i + 1] - bounds[i]) * P
        nc.gpsimd.collective_compute(
            "AllGather", mybir.AluOpType.bypass, replica_groups=rg,
            ins=[agl[i][:, :].opt()],
            outs=[tab[cores * lo:cores * lo + cores * rows, :].opt()])

    def ag_dst(agl, w):
        i = next(j for j in range(nch) if w < bounds[j + 1])
        r = (w - bounds[i]) * P
        return agl[i], i, r

    win_chunks = {}
    for (w, c0, cols, o16) in chunks:
        win_chunks.setdefault(w, []).append((c0, cols, o16))

    with TileContext(nc) as tc:
        with tc.tile_pool(name="const", bufs=1) as cp, \
             tc.tile_pool(name="xin", bufs=3) as xp, \
             tc.tile_pool(name="work", bufs=4) as wk, \
             tc.tile_pool(name="small", bufs=4) as sm, \
             tc.tile_pool(name="ps", bufs=3, space="PSUM") as pp, \
             tc.tile_pool(name="ps2", bufs=2, space="PSUM") as pp2:

            nc.gpsimd.load_library(mlp)
            ident_b = cp.tile([P, P], BF16, tag="identb")
            make_identity(nc, ident_b[:])
            w1_sb = cp.tile([P, ic, D + 8], BF16, tag="w1")
            nc.sync.dma_start(out=w1_sb[:],
                              in_=w1cat.ap().rearrange("c p f -> p c f"))
            w2_sb = cp.tile([P, D + 8], BF16, tag="w2")
            nc.sync.dma_start(out=w2_sb[:], in_=w2cat[:, :])
            w3_sb = cp.tile([P, out_f + 8], BF16, tag="w3")
            nc.sync.dma_start(out=w3_sb[:], in_=w3cat[:, :])
            b1_sb = cp.tile([P, D], F32, tag="b1")
            nc.sync.dma_start(out=b1_sb[:], in_=b1row.ap().to_broadcast((P, D)))
            b2_sb = cp.tile([P, D], F32, tag="b2")
            nc.sync.dma_start(out=b2_sb[:], in_=b2row.ap().to_broadcast((P, D)))
            b3_sb = cp.tile([P, out_f], F32, tag="b3")
            nc.sync.dma_start(out=b3_sb[:],
                              in_=b3row.ap().to_broadcast((P, out_f)))
            idx_sb = cp.tile([P, 8 * S], I16, tag="idx")
            nc.sync.dma_start(out=idx_sb[:], in_=idx16_d[:, :])
            lew_sb = cp.tile([P, S], BF16, tag="lew")
            nc.sync.dma_start(out=lew_sb[:], in_=logew_d[:, :])

            # ---------------- stage A: layer-1 tables (sharded) ------------
            ald1_all = cp.tile([P, wpc, H], BF16, tag="ald1all")
            for t0 in range(0, wpc, GA):
                xt = xp.tile([P, GA, ic, P], BF16, tag="xt")
                nc.sync.dma_start(out=xt[:], in_=xTt[t0 // GA])
                h_sb = sm.tile([P, GA, TW], BF16, tag="hA")
                for g in range(GA):
                    ps = pp.tile([P, D + 8], F32, tag="acc")
                    for c in range(ic):
                        nc.tensor.matmul(ps[:], lhsT=xt[:, g, c, :],
                                         rhs=w1_sb[:, c, :],
                                         start=(c == 0), stop=(c == ic - 1))
                    nc.scalar.activation(h_sb[:, g, 0:D + H], ps[:, 0:D + H],
                                         AF.Copy)
                    nc.vector.tensor_copy(out=ald1_all[:, t0 + g, :],
                                          in_=ps[:, D + 4:D + 4 + H])
                buf, i, r = ag_dst(ag1, t0)
                dst = buf[r:r + GA * P, :]
                nc.sync.dma_start(
                    out=dst.rearrange("(g p) f -> p g f", p=P), in_=h_sb[:])
                if t0 + GA == bounds[i + 1]:
                    ag_issue(ag1, h1tab, i)
            ald2_all = cp.tile([P, wpc, H], BF16, tag="ald2all")
            ald3_all = cp.tile([P, wpc, 1], BF16, tag="ald3all")

            # ---------------- edge pass ------------------------------------
            def edge_layer(layer, tab, tw, ald_all, F_in, H_l, wnext_sb,
                           F_next, H_n, bias_sb, agh, aldnext, ag_cb=None):
                """One GAT layer over all windows. ald_all: SBUF tile."""
                for w in range(wpc):
                    K = int(Ks[w])
                    off = int(offs[w])
                    r0 = w * P
                    V = wk.tile([P, K, tw], BF16, tag="V")
                    for (c0, cols, o16) in win_chunks[w]:
                        nc.gpsimd.dma_gather(
                            V[:, c0:c0 + cols, :], tab[BASE:npad, :],
                            idx_sb[:, o16:o16 + 8 * cols],
                            128 * cols, 128 * cols, tw)
                    if dbgV is not None and layer == 1 and w == debug_w:
                        nc.sync.dma_start(
                            out=dbgV[:, :],
                            in_=V[:].rearrange("p k f -> p (k f)"))
                    ALS = V[:, :, F_in:F_in + H_l]
                    ald_w = ald_all[:, w, :]
                    # logit = ALS + ald (bcast over K) + logew (bcast over H)
                    logit = wk.tile([P, K, H_l], BF16, tag="logit")
                    ald_b = bass.AP(ald_w.tensor, ald_w.offset,
                                    [ald_w.ap[0], [0, K], [1, H_l]])
                    nc.vector.tensor_add(out=logit[:], in0=ALS, in1=ald_b)
                    lew_ap = lew_sb[:, off:off + K]
                    lew_b = bass.AP(lew_ap.tensor, lew_ap.offset,
                                    [lew_ap.ap[0], [1, K], [0, H_l]])
                    nc.vector.tensor_add(out=logit[:], in0=logit[:],
                                         in1=lew_b)
                    # w = exp(max(0.2*logit, logit)), rounded to bf16 on write
                    wt = wk.tile([P, K, H_l], BF16, tag="wt")
                    nc.vector.scalar_tensor_tensor(
                        out=wt[:], in0=logit[:], scalar=SLOPE, in1=logit[:],
                        op0=ALU.mult, op1=ALU.max)
                    wtb = wk.tile([P, K, H_l], BF16, tag="wtb")
                    nc.scalar.activation(wtb[:], wt[:], AF.Exp)
                    # rhs = [V*w | w]
                    rhs = wk.tile([P, K, F_in + H_l], BF16, tag="rhs")
                    ch = F_in // H_l
                    wrep = bass.AP(wtb.tensor, wtb[:].offset,
                                   [wtb[:].ap[0], [H_l, K], [1, H_l], [0, ch]])
                    nc.vector.tensor_mul(out=rhs[:, :, 0:F_in],
                                         in0=V[:, :, 0:F_in], in1=wrep)
                    nc.vector.tensor_copy(out=rhs[:, :, F_in:F_in + H_l],
                                          in_=wtb[:])
                    acc = pp.tile([P, F_in + H_l], F32, tag="acc")
                    for k in range(K):
                        nc.tensor.matmul(acc[:], lhsT=ident_b[:],
                                         rhs=rhs[:, k, :],
                                         start=(k == 0), stop=(k == K - 1))
                    den = sm.tile([P, H_l], F32, tag="den")
                    nc.vector.tensor_scalar_add(den[:],
                                                acc[:, F_in:F_in + H_l],
                                                1e-16)
                    rec = sm.tile([P, H_l], F32, tag="rec")
                    nc.vector.reciprocal(rec[:], den[:])
                    o = sm.tile([P, F_in], F32, tag="o")
                    rrep = bass.AP(rec.tensor, rec[:].offset,
                                   [rec[:].ap[0], [1, H_l], [0, ch]])
                    nc.vector.tensor_mul(out=o[:], in0=acc[:, 0:F_in],
                                         in1=rrep)
                    nc.vector.tensor_add(out=o[:], in0=o[:], in1=bias_sb[:])
                    if dbgo1 is not None and layer == 1:
                        nc.sync.dma_start(out=dbgo1[r0:r0 + P, :], in_=o[:])
                    if dbgo2 is not None and layer == 2:
                        nc.sync.dma_start(out=dbgo2[r0:r0 + P, :], in_=o[:])
                    if layer < 3:
                        o_b = sm.tile([P, F_in], BF16, tag="ob")
                        nc.scalar.activation(o_b[:], o[:], AF.Relu)
                        # next-layer table rows for this window
                        oT = pp2.tile([P, P], BF16, tag="oT")
                        nc.tensor.transpose(out=oT[:], in_=o_b[:],
                                            identity=ident_b[:])
                        oT_sb = sm.tile([P, P], BF16, tag="oTsb")
                        nc.scalar.activation(oT_sb[:], oT[:], AF.Copy)
                        hn = pp2.tile([P, F_next + 8], F32, tag="hn")
                        nc.tensor.matmul(hn[:], lhsT=oT_sb[:],
                                         rhs=wnext_sb[:, 0:F_next + 8],
                                         start=True, stop=True)
                        twn = TW if layer == 1 else TW3
                        hn_sb = sm.tile([P, twn], BF16, tag="hnsb")
                        nc.scalar.activation(hn_sb[:, 0:F_next],
                                             hn[:, 0:F_next], AF.Copy)
                        nc.vector.tensor_copy(
                            out=hn_sb[:, F_next:F_next + H_n],
                            in_=hn[:, F_next:F_next + H_n])
                        buf, _, r = ag_dst(agh, w)
                        nc.sync.dma_start(out=buf[r:r + P, :], in_=hn_sb[:])
                        nc.vector.tensor_copy(
                            out=aldnext[:, w, :],
                            in_=hn[:, F_next + 4:F_next + 4 + H_n])
                    else:
                        nc.sync.dma_start(out=out_d[r0:r0 + P, :], in_=o[:])
                    if ag_cb is not None and (w + 1) in bounds:
                        ag_cb(bounds.index(w + 1) - 1)

            # layer 1  (ag_cb fires for chunks 0..nch-2 inside the loop;
            # the final chunk fires right after the last window)
            edge_layer(1, h1tab, TW, ald1_all, D, H,
                       w2_sb, D, H, b1_sb, ag2, ald2_all,
                       ag_cb=lambda i: ag_issue(ag2, h2tab, i))
            # layer 2
            edge_layer(2, h2tab, TW, ald2_all, D, H,
                       w3_sb, out_f, heads3, b2_sb, ag3, ald3_all,
                       ag_cb=lambda i: ag_issue(ag3, h3tab, i))
            # layer 3
            edge_layer(3, h3tab, TW3, ald3_all, out_f, heads3,
                       None, 0, 1, b3_sb, None, None)
    nc.finalize()
    return nc


# ----------------------------------------------------------------------------
# host entry point
# ----------------------------------------------------------------------------
def prepare_inputs(x, edge_index, edge_weight, W1, a_src1, a_dst1, b1,
                   W2, a_src2, a_dst2, b2, W3, a_src3, a_dst3, b3,
                   npad, cores):
    """Returns (in_maps, perm, Ks, offs, chunks)."""
    x = np.asarray(x, np.float32)
    W1 = np.asarray(W1, np.float32)
    W2 = np.asarray(W2, np.float32)
    W3 = np.asarray(W3, np.float32)
    n_nodes, in_f = x.shape
    d1 = W1.shape[1]
    out_f = W3.shape[1]
    heads = np.asarray(a_src1).shape[0]
    hid = d1 // heads

    perm, Ks, offs, chunks, idx16, logew = build_schedule(
        edge_index[0], edge_index[1], edge_weight, n_nodes, npad, cores)

    xp = np.zeros((npad, in_f), np.float32)
    xp[perm[:n_nodes]] = x
    nc_rows_ = npad // cores
    wpc_ = nc_rows_ // P
    GA = next(g for g in (7, 4, 2, 1) if wpc_ % g == 0)
    ic = in_f // P
    # per-core transposed x: [core, grp, p, g*c*n]
    A = xp.reshape(cores, wpc_ // GA, GA, P, ic, P)   # [c, grp, g, n, ch, p]
    xTt_pc = _np_bf16(np.ascontiguousarray(
        A.transpose(0, 1, 5, 2, 4, 3)).reshape(cores, wpc_ // GA, P,
                                               GA * in_f))

    def wcat(W, a_s, a_d, h, c):
        wa = (W.reshape(W.shape[0], h, c) * np.asarray(a_s)[None]).sum(-1)
        wd = (W.reshape(W.shape[0], h, c) * np.asarray(a_d)[None]).sum(-1)
        pad = np.zeros((W.shape[0], 4 - wa.shape[1]), np.float32)
        return np.concatenate([W, wa, pad, wd, pad], axis=1)

    w1full = wcat(W1, a_src1, a_dst1, heads, hid)          # [256, 136]
    w1cat = _np_bf16(w1full.reshape(2, P, d1 + 8))
    w2cat = _np_bf16(wcat(W2, a_src2, a_dst2, heads, hid))  # [128, 136]
    w3cat = _np_bf16(wcat(W3, a_src3, a_dst3, 1, out_f))    # [128, 72]

    in_maps = []
    for c in range(cores):
        in_maps.append(dict(
            xTt=xTt_pc[c], w1cat=w1cat, w2cat=w2cat, w3cat=w3cat,
            b1row=np.asarray(b1, np.float32).reshape(1, -1),
            b2row=np.asarray(b2, np.float32).reshape(1, -1),
            b3row=np.asarray(b3, np.float32).reshape(1, -1),
            idx16=idx16[c], logew=_np_bf16(logew[c]),
        ))
    return in_maps, perm, Ks, offs, chunks


def kernel(**inputs):
    npad = 50176
    in_maps, perm, Ks, offs, chunks = prepare_inputs(
        npad=npad, cores=CORES, **inputs)
    S = int(offs[-1])
    nc = build_program(npad, Ks, offs, chunks, S, IN, HEADS * HID, OUT, 1,
                       CORES)

    from concourse.bass_utils import run_bass_kernel_spmd
    res = run_bass_kernel_spmd(nc, in_maps, core_ids=list(range(CORES)))
    shards = [res.results[c]["out"] for c in range(CORES)]
    full = np.concatenate(shards, axis=0)       # [npad, OUT] in new-id order
    return full[perm[:N]].astype(np.float32)
